# revision 24
# baseline (speedup 1.0000x reference)
"""LoRA linear (y = x @ (W + s*B@A)^T + bias) on 8 Trainium2 NeuronCores.

Strategy: data parallel over tokens (2048/core), LoRA folded into W on host
(W' = W + 4*B@A), single fp16 GEMM per core.

Why fp16: the PE streams ~1 moving column/cycle (~2.4 GHz warm) regardless
of operand dtype (fp32r == bf16 == fp16 == fp8 at ~216-236 ns per 512-col
instruction), so a single GEMM is 131072 columns ~ 55-58 us steady no
matter what. fp8 DoubleRow halves instructions per GEMM but e4m3 alone
fails the 2e-2 gate (3.8e-2 measured), and the hi+lo correction terms bring
the column count right back to 131072 - same speed, worse error. fp16
keeps full accuracy (fro err 3.6e-4 measured end-to-end) and halves HBM
traffic vs fp32r (x 4 MiB, w 2 MiB, out 4 MiB per core).

Orientation: w stationary ([128d, 128o] slices of one resident SBUF
tensor), x moving ([128d, 512n] fp16), psum [128o, 512n] - bias lands
per-PARTITION so the eviction is ONE fused pass: scalar-engine
activation(Identity, bias=bias[o]) or DVE tensor_scalar(add bias[o]),
alternated per tile so the tail drains on two engines in parallel.
Output leaves as f16 [1024o, 2048n]; host transposes back.

DMA: descriptor issue costs ~610 ns per DMA instruction on a sequencer
(transfers run async), so the fill must minimize instruction count and
split issue across queues (only sync/scalar/gpsimd can DMA): w streams on
sync as 9 per-k chunks (first k split so matmul #1 waits on ~192 KB), x
streams on scalar as 8 per-k chunks for n-block 0 then one 1 MiB
instruction per later n-block (host layout makes each n-block one
contiguous 8 KB/partition run). Out rides gpsimd/sync alternately; bias
rides gpsimd. A dummy Identity activation pre-loads the ACT table off
the critical path. The PE clock ramps over ~3 us of continuous busy
(0.65 -> 1.2 -> 2.4 GHz), and idle gaps reset the ramp: tiny fp32 filler
matmuls on bias_sb (the earliest-arriving tensor, ~25 ns each) plus
fillers on the first-arrived w/x slices keep the PE continuously busy
from ~8 us until real operands land, so every real matmul runs at full
clock. (Big zeroed warm-up tiles were tried and dropped: their memset
dependencies stall the PE and each stall resets the ramp.)

Schedule per core: 4 n-blocks x 8 o-tile psums x 8 k-steps. N-block 0
runs k-step-outer (each arriving (w,x) k-slice enables 8 matmuls during
the fill); blocks 1-3 run o-tile-outer so psum completions and evictions
spread across each block (banks recycle a full block-period later) instead
of bursting at block boundaries.
"""

import os
import sys

import numpy as np

for _p in ("/opt/trn_rl_repo", "/opt/pypackages"):
    if os.path.isdir(_p) and _p not in sys.path:
        sys.path.append(_p)

try:
    import jax

    jax.config.update(
        "jax_compilation_cache_dir", os.path.expanduser("~/.cache/jax_bass_cache")
    )
    jax.config.update("jax_persistent_cache_min_compile_time_secs", 0.0)
except Exception:
    pass

try:
    # bass_utils imports this when tracing is requested via BASS_TRACE; the
    # agent image ships a stub antenv without it. Register a no-op fallback
    # so a trace request degrades to "no trace" instead of crashing.
    from antenv import axon_hooks as _axon_hooks  # noqa: F401
except ImportError:
    import types as _types

    import antenv as _antenv

    _hooks = _types.ModuleType("antenv.axon_hooks")
    _hooks._hook = None
    _hooks.set_axon_ntff_profile_hook = lambda h: setattr(_hooks, "_hook", h)
    _hooks.get_axon_ntff_profile_hook = lambda: _hooks._hook
    sys.modules["antenv.axon_hooks"] = _hooks
    _antenv.axon_hooks = _hooks

import concourse.bass as bass  # noqa: E402,F401
import concourse.mybir as mybir  # noqa: E402
import concourse.tile as tile  # noqa: E402
from concourse import bacc  # noqa: E402
from concourse.bass_utils import run_bass_kernel_spmd  # noqa: E402

N_CORES = 8
N_TOK, D_IN, D_OUT = 16384, 1024, 1024
NS = N_TOK // N_CORES  # 2048 tokens per core
P = 128
KT = D_IN // P  # 8 k-tiles of 128
NB = NS // 512  # 4 n-blocks of 512 tokens
OT = D_OUT // P  # 8 o-tiles
NW = 512  # moving cols per instruction (one PSUM bank)
SCALING = 4.0  # alpha / r = 32 / 8

_CACHE: dict = {}


def build_nc():
    f32 = mybir.dt.float32
    f16 = mybir.dt.float16
    nc = bacc.Bacc("TRN2", target_bir_lowering=False, debug=False)

    # x: per n-block contiguous [128p, KT*NW]; w: [128p, KT*D_OUT]
    x_d = nc.dram_tensor("xT", [P, NB, KT * NW], f16, kind="ExternalInput")
    w_d = nc.dram_tensor("wT", [P, KT * D_OUT], f16, kind="ExternalInput")
    bias_d = nc.dram_tensor("biasT", [P, OT], f32, kind="ExternalInput")
    out_d = nc.dram_tensor("outT", [D_OUT, NS], f16, kind="ExternalOutput")

    with tile.TileContext(nc) as tc:
        with tc.tile_pool(name="const", bufs=1) as cp, \
                tc.tile_pool(name="xp", bufs=NB) as xp, \
                tc.tile_pool(name="op", bufs=8) as op, \
                tc.tile_pool(name="ps", bufs=8, space="PSUM") as pp:
            w_sb = cp.tile([P, KT * D_OUT], f16, name="w")
            bias_sb = cp.tile([P, OT], f32)

            warm_o = cp.tile([P, 1], f32)

            # bias is the FIRST instruction on the otherwise-idle scalar
            # queue: tiny (4 KB), and its completion semaphore gates the PE
            # ramp fillers - cold DMA engines take ~1.4 us to signal even
            # small transfers, and it must not displace w/x on sync.
            nc.scalar.dma_start(bias_sb[:], bias_d[:])

            # ALL x/w input DMAs ride ONE queue (sync) in exact consumption
            # order: per-core DMA bandwidth is shared and the engines run
            # slow (~100 GB/s) for the first ~10 us, so extra input queues
            # only move bandwidth to far-future chunks and starve the chunk
            # the PE needs next (measured: striping across queues stalls
            # the PE 3-4 us). Single queue => transfer order == consumption
            # order.
            x_sb = [xp.tile([P, KT * NW], f16, name=f"x{nb}") for nb in range(NB)]
            nc.sync.dma_start(w_sb[:, 0:P], w_d[:, 0:P])
            nc.sync.dma_start(x_sb[0][:, 0:NW // 2], x_d[:, 0, 0:NW // 2])
            nc.sync.dma_start(x_sb[0][:, NW // 2:NW], x_d[:, 0, NW // 2:NW])
            nc.sync.dma_start(w_sb[:, P:D_OUT], w_d[:, P:D_OUT])
            for k in range(1, KT):
                nc.sync.dma_start(
                    w_sb[:, k * D_OUT:(k + 1) * D_OUT],
                    w_d[:, k * D_OUT:(k + 1) * D_OUT],
                )
                nc.sync.dma_start(
                    x_sb[0][:, k * NW:(k + 1) * NW],
                    x_d[:, 0, k * NW:(k + 1) * NW],
                )
            H = KT * NW // 2
            nc.sync.dma_start(x_sb[1][:, 0:H], x_d[:, 1, 0:H])
            nc.sync.dma_start(x_sb[1][:, H:2 * H], x_d[:, 1, H:2 * H])
            for nb in range(2, NB):
                nc.sync.dma_start(x_sb[nb][:], x_d[:, nb, :])
            # dummy ACT pre-loads the Identity table off the critical path
            nc.scalar.activation(
                warm_o[:], bias_sb[:, 0:1],
                mybir.ActivationFunctionType.Identity, bias=0.0, scale=1.0,
            )

            def evict(nb, j, psum, out_q):
                o_sb = op.tile([P, NW], f16)
                if j % 2 == 0:
                    nc.scalar.activation(
                        o_sb[:], psum[:],
                        mybir.ActivationFunctionType.Identity,
                        bias=bias_sb[:, j:j + 1], scale=1.0,
                    )
                else:
                    nc.vector.tensor_scalar(
                        o_sb[:], psum[:], bias_sb[:, j:j + 1], None,
                        mybir.AluOpType.add,
                    )
                out_q.dma_start(
                    out_d[j * P:(j + 1) * P, nb * NW:(nb + 1) * NW], o_sb[:]
                )

            def w_slice(k, j):
                return w_sb[:, k * D_OUT + j * P:k * D_OUT + (j + 1) * P]

            # bias (4 KB on gpsimd) lands ~1.4 us before the first w/x
            # chunks: tiny fp32 filler matmuls on it start the PE clock
            # ramp early instead of idling until real data arrives.
            warm_ps = pp.tile([P, P], f32, name="warm_ps", tag="psum")
            for _ in range(30):
                nc.tensor.matmul(warm_ps[0:OT, 0:OT], bias_sb[:],
                                 bias_sb[:], start=True, stop=True)

            for nb in range(NB):
                psums = [
                    pp.tile([P, NW], f32, name=f"ps_n{nb}_{j}", tag="psum")
                    for j in range(OT)
                ]
                xv = x_sb[nb]
                if nb == 0:
                    # k-step-outer: each arriving (w, x) k-slice immediately
                    # enables 8 matmuls while later slices are in flight.
                    # After the very first matmul (needs only w[:, 0:128] +
                    # x n-block 0 k0), filler matmuls on that same data keep
                    # the PE busy (and its clock ramp alive) while the rest
                    # of w k0 streams in on the slow early DMA engines.
                    for k in range(KT):
                        for j in range(OT):
                            if k == 0 and j == 0:
                                # first matmul in 256-col halves with
                                # fillers interleaved: keeps the PE (and
                                # its clock ramp) busy while w k0 j1-7
                                # streams in on the slow cold DMA engines
                                nc.tensor.matmul(
                                    psums[0][:, 0:NW // 2],
                                    w_slice(0, 0), xv[:, 0:NW // 2],
                                    start=True, stop=False,
                                )
                                for _ in range(4):
                                    nc.tensor.matmul(
                                        warm_ps[:], w_sb[:, 0:P],
                                        xv[:, 0:P], start=True, stop=True)
                                # start=False: half0's start already
                                # zeroed the whole bank (2 KB zero-region)
                                nc.tensor.matmul(
                                    psums[0][:, NW // 2:NW],
                                    w_slice(0, 0), xv[:, NW // 2:NW],
                                    start=False, stop=False,
                                )
                                for _ in range(8):
                                    nc.tensor.matmul(
                                        warm_ps[:], w_sb[:, 0:P],
                                        xv[:, 0:P], start=True, stop=True)
                                continue
                            nc.tensor.matmul(
                                psums[j][:], w_slice(k, j),
                                xv[:, k * NW:(k + 1) * NW],
                                start=(k == 0 and j != 0), stop=(k == KT - 1),
                            )
                    for j in range(OT):
                        evict(nb, j, psums[j], nc.gpsimd)
                else:
                    # data resident: o-tile-outer spreads psum completions
                    # and evictions across the block, so psum banks recycle
                    # exactly one block-period later (no turnaround stall)
                    # and the tail isn't 8 serialized evictions.
                    for j in range(OT):
                        for k in range(KT):
                            nc.tensor.matmul(
                                psums[j][:], w_slice(k, j),
                                xv[:, k * NW:(k + 1) * NW],
                                start=(k == 0), stop=(k == KT - 1),
                            )
                        if j == OT - 1:
                            # final tile: one ACT pass (two readers of one
                            # psum serialize anyway), out split across two
                            # idle queues so issue+transfer parallelize
                            o_sb = op.tile([P, NW], f16)
                            Hc = NW // 2
                            nc.scalar.activation(
                                o_sb[:], psums[j][:],
                                mybir.ActivationFunctionType.Identity,
                                bias=bias_sb[:, j:j + 1], scale=1.0,
                            )
                            nc.sync.dma_start(
                                out_d[j * P:(j + 1) * P,
                                      nb * NW:nb * NW + Hc],
                                o_sb[:, 0:Hc])
                            nc.gpsimd.dma_start(
                                out_d[j * P:(j + 1) * P,
                                      nb * NW + Hc:(nb + 1) * NW],
                                o_sb[:, Hc:NW])
                        else:
                            # input stream on sync is long done: out DMAs
                            # ride sync/gpsimd so the scalar queue only
                            # runs ACT evictions at the tail
                            evict(nb, j, psums[j],
                                  nc.gpsimd if j % 2 else nc.sync)

    nc.finalize()
    return nc


def _get_nc():
    if "nc" not in _CACHE:
        _CACHE["nc"] = build_nc()
    return _CACHE["nc"]


def kernel(x, weight, bias, A, B):
    x = np.asarray(x, dtype=np.float32)
    weight = np.asarray(weight, dtype=np.float32)
    bias = np.asarray(bias, dtype=np.float32)
    A = np.asarray(A, dtype=np.float32)
    B = np.asarray(B, dtype=np.float32)

    # Fold the rank-8 LoRA update into the weight (exact up to fp32 rounding).
    w_eff = (
        weight.astype(np.float64)
        + SCALING * (B.astype(np.float64) @ A.astype(np.float64))
    ).astype(np.float32)

    # Device layouts (d = k*128 + p):
    #   x [P, NB, KT*NW]: x_host[p, nb, k*512+nn] = x[c*2048+nb*512+nn, k*128+p]
    #   w [P, KT*D_OUT]:  w_host[p, k*1024+o]     = w_eff[o, k*128+p]
    xT16 = np.ascontiguousarray(x.T).astype(np.float16)  # [d, n]
    wT16 = np.ascontiguousarray(w_eff.T).astype(np.float16)  # [d, o]
    # w: [KT, P, D_OUT] -> [P, KT, D_OUT]
    w_host = np.ascontiguousarray(
        wT16.reshape(KT, P, D_OUT).transpose(1, 0, 2).reshape(P, KT * D_OUT))
    biasT = np.ascontiguousarray(bias.reshape(OT, P).T.astype(np.float32))

    def core_x(c):
        v = xT16[:, c * NS:(c + 1) * NS]  # [d, 2048]
        v = v.reshape(KT, P, NB, NW).transpose(1, 2, 0, 3)  # [P, NB, KT, NW]
        return np.ascontiguousarray(v.reshape(P, NB, KT * NW))

    nc = _get_nc()
    in_maps = [
        {"xT": core_x(c), "wT": w_host, "biasT": biasT}
        for c in range(N_CORES)
    ]
    trace_kwargs = {}
    if os.environ.get("KERNEL_TRACE") == "1":
        trace_kwargs = {"trace": True}
    res = run_bass_kernel_spmd(nc, in_maps, list(range(N_CORES)), **trace_kwargs)
    _CACHE["last_results"] = res
    return np.concatenate(
        [r["outT"].astype(np.float32).T for r in res.results], axis=0
    )


# revision 25
# speedup vs baseline: 1.0312x; 1.0312x over previous
"""LoRA linear (y = x @ (W + s*B@A)^T + bias) on 8 Trainium2 NeuronCores.

Strategy: data parallel over tokens (2048/core), LoRA folded into W on host
(W' = W + 4*B@A), single fp16 GEMM per core.

Why fp16: the PE streams ~1 moving column/cycle (~2.4 GHz warm) regardless
of operand dtype (fp32r == bf16 == fp16 == fp8 at ~216-236 ns per 512-col
instruction), so a single GEMM is 131072 columns ~ 55-58 us steady no
matter what. fp8 DoubleRow halves instructions per GEMM but e4m3 alone
fails the 2e-2 gate (3.8e-2 measured), and the hi+lo correction terms bring
the column count right back to 131072 - same speed, worse error. fp16
keeps full accuracy (fro err 3.6e-4 measured end-to-end) and halves HBM
traffic vs fp32r (x 4 MiB, w 2 MiB, out 4 MiB per core).

Orientation: w stationary ([128d, 128o] slices of one resident SBUF
tensor), x moving ([128d, 512n] fp16), psum [128o, 512n] - bias lands
per-PARTITION so the eviction is ONE fused pass: scalar-engine
activation(Identity, bias=bias[o]) or DVE tensor_scalar(add bias[o]),
alternated per tile so the tail drains on two engines in parallel.
Output leaves as f16 [1024o, 2048n]; host transposes back.

DMA: descriptor issue costs ~610 ns per DMA instruction on a sequencer
(transfers run async), so the fill must minimize instruction count and
split issue across queues (only sync/scalar/gpsimd can DMA): w streams on
sync as 9 per-k chunks (first k split so matmul #1 waits on ~192 KB), x
streams on scalar as 8 per-k chunks for n-block 0 then one 1 MiB
instruction per later n-block (host layout makes each n-block one
contiguous 8 KB/partition run). Out rides gpsimd/sync alternately; bias
rides gpsimd. A dummy Identity activation pre-loads the ACT table off
the critical path. The PE clock ramps over ~3 us of continuous busy
(0.65 -> 1.2 -> 2.4 GHz), and idle gaps reset the ramp: tiny fp32 filler
matmuls on bias_sb (the earliest-arriving tensor, ~25 ns each) plus
fillers on the first-arrived w/x slices keep the PE continuously busy
from ~8 us until real operands land, so every real matmul runs at full
clock. (Big zeroed warm-up tiles were tried and dropped: their memset
dependencies stall the PE and each stall resets the ramp.)

Schedule per core: 4 n-blocks x 8 o-tile psums x 8 k-steps. N-block 0
runs k-step-outer (each arriving (w,x) k-slice enables 8 matmuls during
the fill); blocks 1-3 run o-tile-outer so psum completions and evictions
spread across each block (banks recycle a full block-period later) instead
of bursting at block boundaries.
"""

import os
import sys

import numpy as np

for _p in ("/opt/trn_rl_repo", "/opt/pypackages"):
    if os.path.isdir(_p) and _p not in sys.path:
        sys.path.append(_p)

try:
    import jax

    jax.config.update(
        "jax_compilation_cache_dir", os.path.expanduser("~/.cache/jax_bass_cache")
    )
    jax.config.update("jax_persistent_cache_min_compile_time_secs", 0.0)
except Exception:
    pass

try:
    # bass_utils imports this when tracing is requested via BASS_TRACE; the
    # agent image ships a stub antenv without it. Register a no-op fallback
    # so a trace request degrades to "no trace" instead of crashing.
    from antenv import axon_hooks as _axon_hooks  # noqa: F401
except ImportError:
    import types as _types

    import antenv as _antenv

    _hooks = _types.ModuleType("antenv.axon_hooks")
    _hooks._hook = None
    _hooks.set_axon_ntff_profile_hook = lambda h: setattr(_hooks, "_hook", h)
    _hooks.get_axon_ntff_profile_hook = lambda: _hooks._hook
    sys.modules["antenv.axon_hooks"] = _hooks
    _antenv.axon_hooks = _hooks

import concourse.bass as bass  # noqa: E402,F401
import concourse.mybir as mybir  # noqa: E402
import concourse.tile as tile  # noqa: E402
from concourse import bacc  # noqa: E402
from concourse.bass_utils import run_bass_kernel_spmd  # noqa: E402

N_CORES = 8
N_TOK, D_IN, D_OUT = 16384, 1024, 1024
NS = N_TOK // N_CORES  # 2048 tokens per core
P = 128
KT = D_IN // P  # 8 k-tiles of 128
NB = NS // 512  # 4 n-blocks of 512 tokens
OT = D_OUT // P  # 8 o-tiles
NW = 512  # moving cols per instruction (one PSUM bank)
SCALING = 4.0  # alpha / r = 32 / 8

_CACHE: dict = {}


def build_nc():
    f32 = mybir.dt.float32
    f16 = mybir.dt.float16
    nc = bacc.Bacc("TRN2", target_bir_lowering=False, debug=False)

    # x: per n-block contiguous [128p, KT*NW]; w: [128p, KT*D_OUT]
    x_d = nc.dram_tensor("xT", [P, NB, KT * NW], f16, kind="ExternalInput")
    w_d = nc.dram_tensor("wT", [P, KT * D_OUT], f16, kind="ExternalInput")
    bias_d = nc.dram_tensor("biasT", [P, OT], f32, kind="ExternalInput")
    out_d = nc.dram_tensor("outT", [D_OUT, NS], f16, kind="ExternalOutput")

    with tile.TileContext(nc) as tc:
        with tc.tile_pool(name="const", bufs=1) as cp, \
                tc.tile_pool(name="xp", bufs=NB) as xp, \
                tc.tile_pool(name="op", bufs=8) as op, \
                tc.tile_pool(name="ps", bufs=8, space="PSUM") as pp:
            w_sb = cp.tile([P, KT * D_OUT], f16, name="w")
            bias_sb = cp.tile([P, OT], f32)

            warm_o = cp.tile([P, 1], f32)

            # bias is the FIRST instruction on the otherwise-idle scalar
            # queue: tiny (4 KB), and its completion semaphore gates the PE
            # ramp fillers - cold DMA engines take ~1.4 us to signal even
            # small transfers, and it must not displace w/x on sync.
            nc.scalar.dma_start(bias_sb[:], bias_d[:])

            # ALL x/w input DMAs ride ONE queue (sync) in exact consumption
            # order: per-core DMA bandwidth is shared and the engines run
            # slow (~100 GB/s) for the first ~10 us, so extra input queues
            # only move bandwidth to far-future chunks and starve the chunk
            # the PE needs next (measured: striping across queues stalls
            # the PE 3-4 us). Single queue => transfer order == consumption
            # order.
            x_sb = [xp.tile([P, KT * NW], f16, name=f"x{nb}") for nb in range(NB)]
            # 32-byte dummy DMA absorbs the sync engine's ~1.4 us cold-start
            # so the real first chunks signal completion fast
            dummy_sink = cp.tile([P, OT], f32, name="dummy_sink")
            nc.sync.dma_start(dummy_sink[0:1, :], bias_d[0:1, :])
            nc.sync.dma_start(w_sb[:, 0:P], w_d[:, 0:P])
            nc.sync.dma_start(x_sb[0][:, 0:NW // 2], x_d[:, 0, 0:NW // 2])
            nc.sync.dma_start(x_sb[0][:, NW // 2:NW], x_d[:, 0, NW // 2:NW])
            nc.sync.dma_start(w_sb[:, P:D_OUT], w_d[:, P:D_OUT])
            for k in range(1, KT):
                nc.sync.dma_start(
                    w_sb[:, k * D_OUT:(k + 1) * D_OUT],
                    w_d[:, k * D_OUT:(k + 1) * D_OUT],
                )
                nc.sync.dma_start(
                    x_sb[0][:, k * NW:(k + 1) * NW],
                    x_d[:, 0, k * NW:(k + 1) * NW],
                )
            H = KT * NW // 2
            nc.sync.dma_start(x_sb[1][:, 0:H], x_d[:, 1, 0:H])
            nc.sync.dma_start(x_sb[1][:, H:2 * H], x_d[:, 1, H:2 * H])
            for nb in range(2, NB):
                nc.sync.dma_start(x_sb[nb][:], x_d[:, nb, :])
            # dummy ACT pre-loads the Identity table off the critical path
            nc.scalar.activation(
                warm_o[:], bias_sb[:, 0:1],
                mybir.ActivationFunctionType.Identity, bias=0.0, scale=1.0,
            )

            def evict(nb, j, psum, out_q):
                o_sb = op.tile([P, NW], f16)
                if j % 2 == 0:
                    nc.scalar.activation(
                        o_sb[:], psum[:],
                        mybir.ActivationFunctionType.Identity,
                        bias=bias_sb[:, j:j + 1], scale=1.0,
                    )
                else:
                    nc.vector.tensor_scalar(
                        o_sb[:], psum[:], bias_sb[:, j:j + 1], None,
                        mybir.AluOpType.add,
                    )
                out_q.dma_start(
                    out_d[j * P:(j + 1) * P, nb * NW:(nb + 1) * NW], o_sb[:]
                )

            def w_slice(k, j):
                return w_sb[:, k * D_OUT + j * P:k * D_OUT + (j + 1) * P]

            # bias (4 KB on gpsimd) lands ~1.4 us before the first w/x
            # chunks: tiny fp32 filler matmuls on it start the PE clock
            # ramp early instead of idling until real data arrives.
            warm_ps = pp.tile([P, P], f32, name="warm_ps", tag="psum")
            for _ in range(30):
                nc.tensor.matmul(warm_ps[0:OT, 0:OT], bias_sb[:],
                                 bias_sb[:], start=True, stop=True)

            for nb in range(NB):
                psums = [
                    pp.tile([P, NW], f32, name=f"ps_n{nb}_{j}", tag="psum")
                    for j in range(OT)
                ]
                xv = x_sb[nb]
                if nb == 0:
                    # k-step-outer: each arriving (w, x) k-slice immediately
                    # enables 8 matmuls while later slices are in flight.
                    # After the very first matmul (needs only w[:, 0:128] +
                    # x n-block 0 k0), filler matmuls on that same data keep
                    # the PE busy (and its clock ramp alive) while the rest
                    # of w k0 streams in on the slow early DMA engines.
                    for k in range(KT):
                        for j in range(OT):
                            if k == 0 and j == 0:
                                # first matmul in 256-col halves with
                                # fillers interleaved: keeps the PE (and
                                # its clock ramp) busy while w k0 j1-7
                                # streams in on the slow cold DMA engines
                                nc.tensor.matmul(
                                    psums[0][:, 0:NW // 2],
                                    w_slice(0, 0), xv[:, 0:NW // 2],
                                    start=True, stop=False,
                                )
                                for _ in range(4):
                                    nc.tensor.matmul(
                                        warm_ps[:], w_sb[:, 0:P],
                                        xv[:, 0:P], start=True, stop=True)
                                # start=False: half0's start already
                                # zeroed the whole bank (2 KB zero-region)
                                nc.tensor.matmul(
                                    psums[0][:, NW // 2:NW],
                                    w_slice(0, 0), xv[:, NW // 2:NW],
                                    start=False, stop=False,
                                )
                                for _ in range(8):
                                    nc.tensor.matmul(
                                        warm_ps[:], w_sb[:, 0:P],
                                        xv[:, 0:P], start=True, stop=True)
                                continue
                            nc.tensor.matmul(
                                psums[j][:], w_slice(k, j),
                                xv[:, k * NW:(k + 1) * NW],
                                start=(k == 0 and j != 0), stop=(k == KT - 1),
                            )
                    for j in range(OT):
                        evict(nb, j, psums[j], nc.gpsimd)
                else:
                    # data resident: o-tile-outer spreads psum completions
                    # and evictions across the block, so psum banks recycle
                    # exactly one block-period later (no turnaround stall)
                    # and the tail isn't 8 serialized evictions.
                    for j in range(OT):
                        for k in range(KT):
                            nc.tensor.matmul(
                                psums[j][:], w_slice(k, j),
                                xv[:, k * NW:(k + 1) * NW],
                                start=(k == 0), stop=(k == KT - 1),
                            )
                        if j == OT - 1:
                            # final tile: one ACT pass (two readers of one
                            # psum serialize anyway), out split across two
                            # idle queues so issue+transfer parallelize
                            o_sb = op.tile([P, NW], f16)
                            Hc = NW // 2
                            nc.scalar.activation(
                                o_sb[:], psums[j][:],
                                mybir.ActivationFunctionType.Identity,
                                bias=bias_sb[:, j:j + 1], scale=1.0,
                            )
                            nc.sync.dma_start(
                                out_d[j * P:(j + 1) * P,
                                      nb * NW:nb * NW + Hc],
                                o_sb[:, 0:Hc])
                            nc.gpsimd.dma_start(
                                out_d[j * P:(j + 1) * P,
                                      nb * NW + Hc:(nb + 1) * NW],
                                o_sb[:, Hc:NW])
                        else:
                            # input stream on sync is long done: out DMAs
                            # ride sync/gpsimd so the scalar queue only
                            # runs ACT evictions at the tail
                            evict(nb, j, psums[j],
                                  nc.gpsimd if j % 2 else nc.sync)

    nc.finalize()
    return nc


def _get_nc():
    if "nc" not in _CACHE:
        _CACHE["nc"] = build_nc()
    return _CACHE["nc"]


def kernel(x, weight, bias, A, B):
    x = np.asarray(x, dtype=np.float32)
    weight = np.asarray(weight, dtype=np.float32)
    bias = np.asarray(bias, dtype=np.float32)
    A = np.asarray(A, dtype=np.float32)
    B = np.asarray(B, dtype=np.float32)

    # Fold the rank-8 LoRA update into the weight (exact up to fp32 rounding).
    w_eff = (
        weight.astype(np.float64)
        + SCALING * (B.astype(np.float64) @ A.astype(np.float64))
    ).astype(np.float32)

    # Device layouts (d = k*128 + p):
    #   x [P, NB, KT*NW]: x_host[p, nb, k*512+nn] = x[c*2048+nb*512+nn, k*128+p]
    #   w [P, KT*D_OUT]:  w_host[p, k*1024+o]     = w_eff[o, k*128+p]
    xT16 = np.ascontiguousarray(x.T).astype(np.float16)  # [d, n]
    wT16 = np.ascontiguousarray(w_eff.T).astype(np.float16)  # [d, o]
    # w: [KT, P, D_OUT] -> [P, KT, D_OUT]
    w_host = np.ascontiguousarray(
        wT16.reshape(KT, P, D_OUT).transpose(1, 0, 2).reshape(P, KT * D_OUT))
    biasT = np.ascontiguousarray(bias.reshape(OT, P).T.astype(np.float32))

    def core_x(c):
        v = xT16[:, c * NS:(c + 1) * NS]  # [d, 2048]
        v = v.reshape(KT, P, NB, NW).transpose(1, 2, 0, 3)  # [P, NB, KT, NW]
        return np.ascontiguousarray(v.reshape(P, NB, KT * NW))

    nc = _get_nc()
    in_maps = [
        {"xT": core_x(c), "wT": w_host, "biasT": biasT}
        for c in range(N_CORES)
    ]
    trace_kwargs = {}
    if os.environ.get("KERNEL_TRACE") == "1":
        trace_kwargs = {"trace": True}
    res = run_bass_kernel_spmd(nc, in_maps, list(range(N_CORES)), **trace_kwargs)
    _CACHE["last_results"] = res
    return np.concatenate(
        [r["outT"].astype(np.float32).T for r in res.results], axis=0
    )
